# revision 20
# baseline (speedup 1.0000x reference)
import os
import sys

sys.path.insert(0, "/opt/trn_rl_repo")

from contextlib import ExitStack

import numpy as np

import concourse.bass as bass
import concourse.tile as tile
from concourse import mybir
from concourse.bass_utils import run_bass_kernel_spmd

B, N, C, H = 1, 256, 128, 4
DH = C // H
KEY_SCALE = DH**-0.5
NCORES = 8
RPC = int(os.getenv("KRPC", str(N // NCORES)))
KLEVEL = int(os.getenv("KLEVEL", "99"))
WITH_BO = True

F32 = mybir.dt.float32
BF16 = mybir.dt.bfloat16

EXP_DT = BF16

_CACHE = {}


def _legalize_multiwaits(nc, max_waits=1):
    n_fix = 0
    for f in nc.m.functions:
        for blk in f.blocks:
            changed = False
            new_insts = []
            for inst in blk.instructions:
                si = inst.sync_info
                ow = list(si.on_wait) if (si is not None and si.on_wait) else []
                if len(ow) > max_waits:
                    head, tail = ow[:-max_waits], ow[-max_waits:]
                    while head:
                        chunk, head = head[:max_waits], head[max_waits:]
                        d = mybir.InstNoOp(
                            name=f"I-mw{nc.next_id()}", ins=[], outs=[]
                        )
                        d.engine = inst.engine
                        d.sync_info = mybir.SyncInfo(
                            on_wait=list(chunk), on_update=[]
                        )
                        new_insts.append(d)
                        n_fix += 1
                    inst.sync_info = mybir.SyncInfo(
                        on_wait=list(tail),
                        on_update=list(si.on_update) if si.on_update else [],
                    )
                    changed = True
                new_insts.append(inst)
            if changed:
                blk.instructions = new_insts
    return n_fix


def _emit(ctx: ExitStack, tc: "tile.TileContext", t):
    nc = tc.nc

    const = ctx.enter_context(tc.tile_pool(name="const", bufs=1))

    def load_const(name, shape, dtype=F32):
        sb = const.tile(shape, dtype, name=name + "_sb")
        nc.sync.dma_start(sb, t[name].ap())
        return sb

    wq_sb = load_const("wqT", [C, C], BF16)
    wk_sb = load_const("wkT", [C, C], BF16)
    wv_sb = load_const("wvT", [C, C], BF16)
    wg_sb = load_const("wgT", [C, C], BF16)
    wo_sb = load_const("woT", [C, C], BF16)
    bo_sb = load_const("bo_row", [1, C], BF16)
    bgn_sb = load_const("bgn_col", [C, 1])
    bias_sb = load_const("bias_r", [128, 2 * RPC])
    nbt_sb = load_const("nbT", [128, 2 * H * N])

    ones1_sb = const.tile([1, C], BF16)
    nc.vector.memset(ones1_sb, 1.0)
    ones32_sb = const.tile([128, DH], EXP_DT)
    nc.vector.memset(ones32_sb, 1.0)

    enb_sb = const.tile([128, 2 * H * N], EXP_DT)
    nc.scalar.activation(enb_sb, nbt_sb, mybir.ActivationFunctionType.Exp)

    io = ctx.enter_context(tc.tile_pool(name="io", bufs=3))
    sb = ctx.enter_context(tc.tile_pool(name="sb", bufs=2))
    exps = ctx.enter_context(tc.tile_pool(name="exps", bufs=3))
    lg_ps = ctx.enter_context(tc.tile_pool(name="lg_ps", bufs=2, space="PSUM"))
    sm_ps = ctx.enter_context(tc.tile_pool(name="sm_ps", bufs=4, space="PSUM"))

    xt_ap = t["xt"].ap()
    mt_ap = t["mt"].ap()
    out_ap = t["out"]

    MM = nc.tensor.matmul
    Act = mybir.ActivationFunctionType
    NB = 512

    def pair_view(tile_ap, width):
        return tile_ap.rearrange("p (b x) -> p b x", b=2)[:, :, 0:width]

    for r in range(RPC):
        xt_sb = io.tile([128, N], BF16, tag="xt")
        nc.sync.dma_start(xt_sb, xt_ap[r])
        mt_sb = io.tile([128, N], BF16, tag="mt")
        nc.sync.dma_start(mt_sb, mt_ap[r])

        q_ps = sm_ps.tile([128, N], F32, tag="sm")
        MM(q_ps, lhsT=wq_sb, rhs=xt_sb, start=True, stop=True)
        k_ps = sm_ps.tile([128, N], F32, tag="sm")
        MM(k_ps, lhsT=wk_sb, rhs=mt_sb, start=True, stop=True)
        qk_sb = sb.tile([128, 2 * N], BF16, tag="qk")
        nc.vector.tensor_copy(qk_sb[:, 0:N], q_ps)
        nc.vector.tensor_copy(qk_sb[:, N : 2 * N], k_ps)

        v0_ps = sm_ps.tile([128, C], F32, tag="sm", padded_shape=[128, N])
        MM(v0_ps, lhsT=mt_sb[:, 0:128], rhs=wv_sb, start=True, stop=True)
        v1_ps = sm_ps.tile([128, C], F32, tag="sm", padded_shape=[128, N])
        MM(v1_ps, lhsT=mt_sb[:, 128:256], rhs=wv_sb, start=True, stop=True)
        v_sb = sb.tile([128, 2 * C], EXP_DT, tag="v")
        nc.vector.tensor_copy(v_sb[:, 0:C], v0_ps)
        nc.vector.tensor_copy(v_sb[:, C : 2 * C], v1_ps)

        g_ps = sm_ps.tile([128, N], F32, tag="sm")
        MM(g_ps, lhsT=wg_sb, rhs=xt_sb, start=True, stop=True)
        e1_sb = sb.tile([128, N], F32, tag="e1")
        nc.scalar.activation(e1_sb, g_ps, Act.Exp, bias=bgn_sb, scale=-1.0)

        wa_ps = sm_ps.tile([128, N], F32, tag="sm")
        s_ps = sm_ps.tile([128, N], F32, tag="sm")
        for kc in range(2):
            for pr in range(2):
                lg = lg_ps.tile([128, 2 * NB], F32, tag="lg", name=f"lg{kc}{pr}")
                for hh in range(2):
                    h = 2 * pr + hh
                    MM(
                        lg[:, NB * hh : NB * hh + N],
                        lhsT=qk_sb[
                            32 * h : 32 * h + 32,
                            N + 128 * kc : N + 128 * kc + 128,
                        ],
                        rhs=qk_sb[32 * h : 32 * h + 32, 0:N],
                        start=True,
                        stop=True,
                        tile_position=(32 * h, 0),
                    )
                e_sb = exps.tile([128, 2, N], EXP_DT, tag="e")
                nc.scalar.activation(
                    e_sb,
                    pair_view(lg, N),
                    Act.Exp,
                    bias=bias_sb[:, kc * RPC + r : kc * RPC + r + 1],
                    scale=KEY_SCALE,
                )
                e_sb = e_sb.rearrange("p b x -> p (b x)")
                nc.vector.tensor_mul(
                    e_sb,
                    e_sb,
                    enb_sb[:, 1024 * kc + 512 * pr : 1024 * kc + 512 * pr + 512],
                )
                for hh in range(2):
                    h = 2 * pr + hh
                    MM(
                        wa_ps[32 * h : 32 * h + 32, :],
                        lhsT=v_sb[:, 128 * kc + 32 * h : 128 * kc + 32 * h + 32],
                        rhs=e_sb[:, N * hh : N * hh + N],
                        start=(kc == 0),
                        stop=(kc == 1),
                        tile_position=(0, 32 * h),
                        skip_group_check=True,
                    )
                    MM(
                        s_ps[32 * h : 32 * h + 32, :],
                        lhsT=ones32_sb,
                        rhs=e_sb[:, N * hh : N * hh + N],
                        start=(kc == 0),
                        stop=(kc == 1),
                        tile_position=(0, 32 * h),
                        skip_group_check=True,
                    )

        d_sb = sb.tile([128, N], F32, tag="d")
        nc.vector.scalar_tensor_tensor(
            d_sb, e1_sb, 1.0, s_ps, mybir.AluOpType.add, mybir.AluOpType.mult
        )
        nc.scalar.activation(d_sb, d_sb, Act.Ln)
        rs_sb = sb.tile([128, N], F32, tag="rs")
        nc.scalar.activation(rs_sb, d_sb, Act.Exp, scale=-1.0)
        wag_sb = sb.tile([128, N], BF16, tag="wag")
        nc.vector.tensor_mul(wag_sb, wa_ps, rs_sb)

        out_sb = sb.tile([128, 2 * C], F32, tag="out")
        for qc in range(2):
            o_ps = sm_ps.tile([128, C], F32, tag="sm", name=f"o{qc}_ps",
                              padded_shape=[128, N])
            MM(
                o_ps,
                lhsT=wag_sb[:, 128 * qc : 128 * qc + 128],
                rhs=wo_sb,
                start=True,
                stop=not WITH_BO,
            )
            if WITH_BO:
                MM(
                    o_ps,
                    lhsT=ones1_sb,
                    rhs=bo_sb,
                    start=False,
                    stop=True,
                    skip_group_check=True,
                )
            nc.vector.tensor_copy(out_sb[:, C * qc : C * qc + C], o_ps)
        dst = bass.AP(out_ap, r * N * C, [[C, 128], [128 * C, 2], [1, C]])
        nc.sync.dma_start(dst, out_sb)


def _build():
    if "nc" in _CACHE:
        return _CACHE["nc"], _CACHE["t"]
    nc = bass.Bass(
        "TRN2", target_bir_lowering=False, debug=False, num_devices=NCORES
    )
    t = {}
    t["xt"] = nc.dram_tensor("xt", [RPC, C, N], BF16, kind="ExternalInput")
    t["mt"] = nc.dram_tensor("mt", [RPC, C, N], BF16, kind="ExternalInput")
    t["bias_r"] = nc.dram_tensor("bias_r", [128, 2 * RPC], F32, kind="ExternalInput")
    t["nbT"] = nc.dram_tensor("nbT", [128, 2 * H * N], F32, kind="ExternalInput")
    for name in ("wqT", "wkT", "wvT", "wgT", "woT"):
        t[name] = nc.dram_tensor(name, [C, C], BF16, kind="ExternalInput")
    t["bo_row"] = nc.dram_tensor("bo_row", [1, C], BF16, kind="ExternalInput")
    t["bgn_col"] = nc.dram_tensor("bgn_col", [C, 1], F32, kind="ExternalInput")
    t["out"] = nc.dram_tensor("out", [RPC, N, C], F32, kind="ExternalOutput")

    with tile.TileContext(nc) as tc:
        with ExitStack() as ctx:
            _emit(ctx, tc, t)
    _legalize_multiwaits(nc, max_waits=1)
    _CACHE["nc"] = nc
    _CACHE["t"] = t
    return nc, t


def _prep_in_maps(q_data, m_data, bias, nonbatched_bias, wq, wk, wv, wo, bo, wg, bg):
    bf16 = mybir.dt.np(BF16)
    q_data = np.ascontiguousarray(np.asarray(q_data, np.float32))
    m_data = np.ascontiguousarray(np.asarray(m_data, np.float32))
    bias = np.asarray(bias, np.float32)
    nb = np.asarray(nonbatched_bias, np.float32)

    consts = {
        "wqT": np.ascontiguousarray(np.asarray(wq, np.float32).T.astype(bf16)),
        "wkT": np.ascontiguousarray(np.asarray(wk, np.float32).T.astype(bf16)),
        "wvT": np.ascontiguousarray(np.asarray(wv, np.float32).T.astype(bf16)),
        "wgT": np.ascontiguousarray(np.asarray(wg, np.float32).T.astype(bf16)),
        "woT": np.ascontiguousarray(np.asarray(wo, np.float32).T.astype(bf16)),
        "bo_row": np.ascontiguousarray(np.asarray(bo, np.float32)[None, :].astype(bf16)),
        "bgn_col": np.ascontiguousarray(
            (-np.asarray(bg, np.float32))[:, None]
        ),
        "nbT": np.ascontiguousarray(
            nb[0]
            .transpose(2, 0, 1)
            .reshape(2, 128, H, N)
            .transpose(1, 0, 2, 3)
            .reshape(128, 2 * H * N)
        ),
    }
    bias_kn = bias[0, :, 0, 0, :].T.reshape(2, 128, N)
    in_maps = []
    for c in range(NCORES):
        n0 = c * RPC
        rows = slice(n0, n0 + RPC)
        m = dict(consts)
        m["xt"] = np.ascontiguousarray(q_data[0, rows].transpose(0, 2, 1).astype(bf16))
        m["mt"] = np.ascontiguousarray(m_data[0, rows].transpose(0, 2, 1).astype(bf16))
        m["bias_r"] = np.ascontiguousarray(
            bias_kn[:, :, rows].transpose(1, 0, 2).reshape(128, 2 * RPC)
        )
        in_maps.append(m)
    return in_maps


def kernel(**inputs) -> np.ndarray:
    global WITH_BO
    want_bo = bool(np.any(np.asarray(inputs["bo"]) != 0))
    if want_bo != WITH_BO or "nc" not in _CACHE:
        WITH_BO = want_bo
        _CACHE.clear()
    nc, _ = _build()
    in_maps = _prep_in_maps(**inputs)
    res = run_bass_kernel_spmd(nc, in_maps, core_ids=list(range(NCORES)))
    out = np.concatenate([res.results[c]["out"] for c in range(NCORES)], axis=0)
    return out.reshape(B, N, N, C).astype(np.float32)


if __name__ == "__main__":
    rng = np.random.default_rng(0)
    inputs = {
        "q_data": rng.standard_normal((B, N, N, C), np.float32),
        "m_data": rng.standard_normal((B, N, N, C), np.float32),
        "bias": rng.standard_normal((B, N, 1, 1, N), np.float32),
        "nonbatched_bias": rng.standard_normal((1, H, N, N), np.float32),
        "wq": rng.standard_normal((C, C), np.float32) / np.sqrt(C),
        "wk": rng.standard_normal((C, C), np.float32) / np.sqrt(C),
        "wv": rng.standard_normal((C, C), np.float32) / np.sqrt(C),
        "wo": rng.standard_normal((C, C), np.float32) / np.sqrt(C),
        "bo": np.zeros((C,), np.float32),
        "wg": rng.standard_normal((C, C), np.float32) / np.sqrt(C),
        "bg": np.ones((C,), np.float32),
    }
    out = kernel(**inputs)
    print("out", out.shape, out.dtype, float(np.abs(out).max()))


# revision 21
# speedup vs baseline: 1.0087x; 1.0087x over previous
import os
import sys

sys.path.insert(0, "/opt/trn_rl_repo")

from contextlib import ExitStack

import numpy as np

import concourse.bass as bass
import concourse.tile as tile
from concourse import mybir
from concourse.bass_utils import run_bass_kernel_spmd

B, N, C, H = 1, 256, 128, 4
DH = C // H
KEY_SCALE = DH**-0.5
NCORES = 8
RPC = int(os.getenv("KRPC", str(N // NCORES)))
KLEVEL = int(os.getenv("KLEVEL", "99"))
WITH_BO = True

F32 = mybir.dt.float32
BF16 = mybir.dt.bfloat16

EXP_DT = BF16

_CACHE = {}


def _legalize_multiwaits(nc, max_waits=1):
    n_fix = 0
    for f in nc.m.functions:
        for blk in f.blocks:
            changed = False
            new_insts = []
            for inst in blk.instructions:
                si = inst.sync_info
                ow = list(si.on_wait) if (si is not None and si.on_wait) else []
                if len(ow) > max_waits:
                    head, tail = ow[:-max_waits], ow[-max_waits:]
                    while head:
                        chunk, head = head[:max_waits], head[max_waits:]
                        d = mybir.InstNoOp(
                            name=f"I-mw{nc.next_id()}", ins=[], outs=[]
                        )
                        d.engine = inst.engine
                        d.sync_info = mybir.SyncInfo(
                            on_wait=list(chunk), on_update=[]
                        )
                        new_insts.append(d)
                        n_fix += 1
                    inst.sync_info = mybir.SyncInfo(
                        on_wait=list(tail),
                        on_update=list(si.on_update) if si.on_update else [],
                    )
                    changed = True
                new_insts.append(inst)
            if changed:
                blk.instructions = new_insts
    return n_fix


def _emit(ctx: ExitStack, tc: "tile.TileContext", t):
    nc = tc.nc

    const = ctx.enter_context(tc.tile_pool(name="const", bufs=1))

    def load_const(name, shape, dtype=F32):
        sb = const.tile(shape, dtype, name=name + "_sb")
        nc.sync.dma_start(sb, t[name].ap())
        return sb

    wq_sb = load_const("wqT", [C, C], BF16)
    wk_sb = load_const("wkT", [C, C], BF16)
    wv_sb = load_const("wvT", [C, C], BF16)
    wg_sb = load_const("wgT", [C, C], BF16)
    wo_sb = load_const("woT", [C, C], BF16)
    bo_sb = load_const("bo_row", [1, C], BF16)
    bgn_sb = load_const("bgn_col", [C, 1])
    bias_sb = load_const("bias_r", [128, 2 * RPC])
    nbt_sb = load_const("nbT", [128, 2 * H * N])

    ones1_sb = const.tile([1, C], BF16)
    nc.vector.memset(ones1_sb, 1.0)
    ones32_sb = const.tile([128, DH], EXP_DT)
    nc.vector.memset(ones32_sb, 1.0)

    enb_sb = const.tile([128, 2 * H * N], EXP_DT)
    nc.scalar.activation(enb_sb, nbt_sb, mybir.ActivationFunctionType.Exp)

    io = ctx.enter_context(tc.tile_pool(name="io", bufs=4))
    sb = ctx.enter_context(tc.tile_pool(name="sb", bufs=3))
    exps = ctx.enter_context(tc.tile_pool(name="exps", bufs=5))
    lg_ps = ctx.enter_context(tc.tile_pool(name="lg_ps", bufs=2, space="PSUM"))
    sm_ps = ctx.enter_context(tc.tile_pool(name="sm_ps", bufs=4, space="PSUM"))

    xt_ap = t["xt"].ap()
    mt_ap = t["mt"].ap()
    out_ap = t["out"]

    MM = nc.tensor.matmul
    Act = mybir.ActivationFunctionType
    NB = 512

    def pair_view(tile_ap, width):
        return tile_ap.rearrange("p (b x) -> p b x", b=2)[:, :, 0:width]

    for r in range(RPC):
        xt_sb = io.tile([128, N], BF16, tag="xt")
        nc.sync.dma_start(xt_sb, xt_ap[r])
        mt_sb = io.tile([128, N], BF16, tag="mt")
        nc.sync.dma_start(mt_sb, mt_ap[r])

        q_ps = sm_ps.tile([128, N], F32, tag="sm")
        MM(q_ps, lhsT=wq_sb, rhs=xt_sb, start=True, stop=True)
        k_ps = sm_ps.tile([128, N], F32, tag="sm")
        MM(k_ps, lhsT=wk_sb, rhs=mt_sb, start=True, stop=True)
        qk_sb = sb.tile([128, 2 * N], BF16, tag="qk")
        nc.vector.tensor_copy(qk_sb[:, 0:N], q_ps)
        nc.vector.tensor_copy(qk_sb[:, N : 2 * N], k_ps)

        v0_ps = sm_ps.tile([128, C], F32, tag="sm", padded_shape=[128, N])
        MM(v0_ps, lhsT=mt_sb[:, 0:128], rhs=wv_sb, start=True, stop=True)
        v1_ps = sm_ps.tile([128, C], F32, tag="sm", padded_shape=[128, N])
        MM(v1_ps, lhsT=mt_sb[:, 128:256], rhs=wv_sb, start=True, stop=True)
        v_sb = sb.tile([128, 2 * C], EXP_DT, tag="v")
        nc.vector.tensor_copy(v_sb[:, 0:C], v0_ps)
        nc.vector.tensor_copy(v_sb[:, C : 2 * C], v1_ps)

        g_ps = sm_ps.tile([128, N], F32, tag="sm")
        MM(g_ps, lhsT=wg_sb, rhs=xt_sb, start=True, stop=True)
        e1_sb = sb.tile([128, N], F32, tag="e1")
        nc.scalar.activation(e1_sb, g_ps, Act.Exp, bias=bgn_sb, scale=-1.0)

        wa_ps = sm_ps.tile([128, N], F32, tag="sm")
        s_ps = sm_ps.tile([128, N], F32, tag="sm")
        for kc in range(2):
            for pr in range(2):
                lg = lg_ps.tile([128, 2 * NB], F32, tag="lg", name=f"lg{kc}{pr}")
                for hh in range(2):
                    h = 2 * pr + hh
                    MM(
                        lg[:, NB * hh : NB * hh + N],
                        lhsT=qk_sb[
                            32 * h : 32 * h + 32,
                            N + 128 * kc : N + 128 * kc + 128,
                        ],
                        rhs=qk_sb[32 * h : 32 * h + 32, 0:N],
                        start=True,
                        stop=True,
                        tile_position=(32 * h, 0),
                    )
                e_sb = exps.tile([128, 2, N], EXP_DT, tag="e")
                nc.scalar.activation(
                    e_sb,
                    pair_view(lg, N),
                    Act.Exp,
                    bias=bias_sb[:, kc * RPC + r : kc * RPC + r + 1],
                    scale=KEY_SCALE,
                )
                e_sb = e_sb.rearrange("p b x -> p (b x)")
                nc.vector.tensor_mul(
                    e_sb,
                    e_sb,
                    enb_sb[:, 1024 * kc + 512 * pr : 1024 * kc + 512 * pr + 512],
                )
                for hh in range(2):
                    h = 2 * pr + hh
                    MM(
                        wa_ps[32 * h : 32 * h + 32, :],
                        lhsT=v_sb[:, 128 * kc + 32 * h : 128 * kc + 32 * h + 32],
                        rhs=e_sb[:, N * hh : N * hh + N],
                        start=(kc == 0),
                        stop=(kc == 1),
                        tile_position=(0, 32 * h),
                        skip_group_check=True,
                    )
                    MM(
                        s_ps[32 * h : 32 * h + 32, :],
                        lhsT=ones32_sb,
                        rhs=e_sb[:, N * hh : N * hh + N],
                        start=(kc == 0),
                        stop=(kc == 1),
                        tile_position=(0, 32 * h),
                        skip_group_check=True,
                    )

        d_sb = sb.tile([128, N], F32, tag="d")
        nc.vector.scalar_tensor_tensor(
            d_sb, e1_sb, 1.0, s_ps, mybir.AluOpType.add, mybir.AluOpType.mult
        )
        nc.scalar.activation(d_sb, d_sb, Act.Ln)
        rs_sb = sb.tile([128, N], F32, tag="rs")
        nc.scalar.activation(rs_sb, d_sb, Act.Exp, scale=-1.0)
        wag_sb = sb.tile([128, N], BF16, tag="wag")
        nc.vector.tensor_mul(wag_sb, wa_ps, rs_sb)

        out_sb = sb.tile([128, 2 * C], F32, tag="out")
        for qc in range(2):
            o_ps = sm_ps.tile([128, C], F32, tag="sm", name=f"o{qc}_ps",
                              padded_shape=[128, N])
            MM(
                o_ps,
                lhsT=wag_sb[:, 128 * qc : 128 * qc + 128],
                rhs=wo_sb,
                start=True,
                stop=not WITH_BO,
            )
            if WITH_BO:
                MM(
                    o_ps,
                    lhsT=ones1_sb,
                    rhs=bo_sb,
                    start=False,
                    stop=True,
                    skip_group_check=True,
                )
            nc.vector.tensor_copy(out_sb[:, C * qc : C * qc + C], o_ps)
        dst = bass.AP(out_ap, r * N * C, [[C, 128], [128 * C, 2], [1, C]])
        nc.sync.dma_start(dst, out_sb)


def _build():
    if "nc" in _CACHE:
        return _CACHE["nc"], _CACHE["t"]
    nc = bass.Bass(
        "TRN2", target_bir_lowering=False, debug=False, num_devices=NCORES
    )
    t = {}
    t["xt"] = nc.dram_tensor("xt", [RPC, C, N], BF16, kind="ExternalInput")
    t["mt"] = nc.dram_tensor("mt", [RPC, C, N], BF16, kind="ExternalInput")
    t["bias_r"] = nc.dram_tensor("bias_r", [128, 2 * RPC], F32, kind="ExternalInput")
    t["nbT"] = nc.dram_tensor("nbT", [128, 2 * H * N], F32, kind="ExternalInput")
    for name in ("wqT", "wkT", "wvT", "wgT", "woT"):
        t[name] = nc.dram_tensor(name, [C, C], BF16, kind="ExternalInput")
    t["bo_row"] = nc.dram_tensor("bo_row", [1, C], BF16, kind="ExternalInput")
    t["bgn_col"] = nc.dram_tensor("bgn_col", [C, 1], F32, kind="ExternalInput")
    t["out"] = nc.dram_tensor("out", [RPC, N, C], F32, kind="ExternalOutput")

    with tile.TileContext(nc) as tc:
        with ExitStack() as ctx:
            _emit(ctx, tc, t)
    _legalize_multiwaits(nc, max_waits=1)
    _CACHE["nc"] = nc
    _CACHE["t"] = t
    return nc, t


def _prep_in_maps(q_data, m_data, bias, nonbatched_bias, wq, wk, wv, wo, bo, wg, bg):
    bf16 = mybir.dt.np(BF16)
    q_data = np.ascontiguousarray(np.asarray(q_data, np.float32))
    m_data = np.ascontiguousarray(np.asarray(m_data, np.float32))
    bias = np.asarray(bias, np.float32)
    nb = np.asarray(nonbatched_bias, np.float32)

    consts = {
        "wqT": np.ascontiguousarray(np.asarray(wq, np.float32).T.astype(bf16)),
        "wkT": np.ascontiguousarray(np.asarray(wk, np.float32).T.astype(bf16)),
        "wvT": np.ascontiguousarray(np.asarray(wv, np.float32).T.astype(bf16)),
        "wgT": np.ascontiguousarray(np.asarray(wg, np.float32).T.astype(bf16)),
        "woT": np.ascontiguousarray(np.asarray(wo, np.float32).T.astype(bf16)),
        "bo_row": np.ascontiguousarray(np.asarray(bo, np.float32)[None, :].astype(bf16)),
        "bgn_col": np.ascontiguousarray(
            (-np.asarray(bg, np.float32))[:, None]
        ),
        "nbT": np.ascontiguousarray(
            nb[0]
            .transpose(2, 0, 1)
            .reshape(2, 128, H, N)
            .transpose(1, 0, 2, 3)
            .reshape(128, 2 * H * N)
        ),
    }
    bias_kn = bias[0, :, 0, 0, :].T.reshape(2, 128, N)
    in_maps = []
    for c in range(NCORES):
        n0 = c * RPC
        rows = slice(n0, n0 + RPC)
        m = dict(consts)
        m["xt"] = np.ascontiguousarray(q_data[0, rows].transpose(0, 2, 1).astype(bf16))
        m["mt"] = np.ascontiguousarray(m_data[0, rows].transpose(0, 2, 1).astype(bf16))
        m["bias_r"] = np.ascontiguousarray(
            bias_kn[:, :, rows].transpose(1, 0, 2).reshape(128, 2 * RPC)
        )
        in_maps.append(m)
    return in_maps


def kernel(**inputs) -> np.ndarray:
    global WITH_BO
    want_bo = bool(np.any(np.asarray(inputs["bo"]) != 0))
    if want_bo != WITH_BO or "nc" not in _CACHE:
        WITH_BO = want_bo
        _CACHE.clear()
    nc, _ = _build()
    in_maps = _prep_in_maps(**inputs)
    res = run_bass_kernel_spmd(nc, in_maps, core_ids=list(range(NCORES)))
    out = np.concatenate([res.results[c]["out"] for c in range(NCORES)], axis=0)
    return out.reshape(B, N, N, C).astype(np.float32)


if __name__ == "__main__":
    rng = np.random.default_rng(0)
    inputs = {
        "q_data": rng.standard_normal((B, N, N, C), np.float32),
        "m_data": rng.standard_normal((B, N, N, C), np.float32),
        "bias": rng.standard_normal((B, N, 1, 1, N), np.float32),
        "nonbatched_bias": rng.standard_normal((1, H, N, N), np.float32),
        "wq": rng.standard_normal((C, C), np.float32) / np.sqrt(C),
        "wk": rng.standard_normal((C, C), np.float32) / np.sqrt(C),
        "wv": rng.standard_normal((C, C), np.float32) / np.sqrt(C),
        "wo": rng.standard_normal((C, C), np.float32) / np.sqrt(C),
        "bo": np.zeros((C,), np.float32),
        "wg": rng.standard_normal((C, C), np.float32) / np.sqrt(C),
        "bg": np.ones((C,), np.float32),
    }
    out = kernel(**inputs)
    print("out", out.shape, out.dtype, float(np.abs(out).max()))
